# revision 6
# baseline (speedup 1.0000x reference)
"""Causal single-head attention (B=2, T=4096, C=1024, D=64) on 8 TRN2 cores.

Sharding: core i -> batch b = i//4, query phase c = i%4: the core owns the
strided query rows {4j + c : j in [0,1024)}. This balances causal work
exactly across cores AND lets each core skip fully-masked key tiles:

  - x[b] is column-permuted on host (within every group of 4 columns,
    rotate by c) so the core's query columns sit at slots 4j — a
    compile-time stride-4 slice, identical on every core.
  - key tile kt (slots 128kt..128kt+127) is attended only by query
    columns j >= 32kt: columns j >= 32(kt+1) are fully visible,
    j in [32kt, 32kt+32) are the diagonal band (one host-computed
    [128,32] 0/1 mask, same for every kt), and j < 32kt are fully
    masked — never computed.
  - denominator comes free from a ones-column in V' (column 64); the
    kernel returns unnormalized [65, 1024] = [PV^T ; rowsum]; host divides.

Two attention phases over descending key tiles (chunks DMA'd in reverse;
chunk ch supplies key tiles 4ch..4ch+3 and query columns [128ch, 128ch+128)):
  phase 1: kt 31..0 on query columns [max(32kt,512), 1024) — only needs
           Q of chunks 4..7, S tiles are <=512 wide (1 PSUM bank), and
           pv[:, 512:] completes here so its output DMA overlaps phase 2.
  phase 2: kt 15..0 on query columns [32kt, 512) — pure compute tail.
Projection work is smeared between attends to keep PE dense; PV
accumulates into a PSUM bank pre-zeroed by a contraction-1 zero matmul.
"""

import numpy as np

B, T, C, D = 2, 4096, 1024, 64
NCORES = 8
TQ = 1024          # queries per core (strided by 4)
NKT = T // 128     # 32 key tiles of 128
DTYPE_NAME = "bfloat16"

_CACHE = {}


def _dtypes():
    import concourse.mybir as mybir
    if DTYPE_NAME == "bfloat16":
        import ml_dtypes
        return mybir.dt.bfloat16, ml_dtypes.bfloat16
    return mybir.dt.float32, np.float32


def _build_program(dt_x):
    import concourse.bass as bass
    import concourse.mybir as mybir
    import concourse.tile as tile
    from concourse import bacc
    from concourse.masks import make_identity
    from contextlib import ExitStack

    f32 = mybir.dt.float32

    nc = bacc.Bacc(
        "TRN2",
        target_bir_lowering=False,
        debug=False,
        num_devices=NCORES,
    )

    xT_t = nc.dram_tensor("xT", [C, T], dt_x, kind="ExternalInput")
    wkv_t = nc.dram_tensor("wkv", [128, 8, 128], dt_x, kind="ExternalInput")
    wq_t = nc.dram_tensor("wq", [128, 8, 64], dt_x, kind="ExternalInput")
    m32_t = nc.dram_tensor("m32", [128, 32], dt_x, kind="ExternalInput")
    out_t = nc.dram_tensor("outT", [65, TQ], f32, kind="ExternalOutput")

    xT = xT_t.ap()
    wkv = wkv_t.ap()
    wq = wq_t.ap()
    m32 = m32_t.ap()
    outT = out_t.ap()

    with tile.TileContext(nc) as tc, ExitStack() as ctx:
        const = ctx.enter_context(tc.tile_pool(name="const", bufs=1))
        xpool = ctx.enter_context(tc.tile_pool(name="xpool", bufs=8))
        stage = ctx.enter_context(tc.tile_pool(name="stage", bufs=3))
        qxp = ctx.enter_context(tc.tile_pool(name="qxp", bufs=2))
        ppool = ctx.enter_context(tc.tile_pool(name="ppool", bufs=4))
        psA = ctx.enter_context(tc.tile_pool(name="psA", bufs=3, space="PSUM"))
        psP = ctx.enter_context(tc.tile_pool(name="psP", bufs=2, space="PSUM"))
        psO = ctx.enter_context(tc.tile_pool(name="psO", bufs=1, space="PSUM"))

        # persistent SBUF tensors
        KT = const.tile([64, T], dt_x)         # K^T, key slot order
        VS = const.tile([128, NKT, 65], dt_x)  # V': [:, kt, 0:64] = V, col 64 = 1
        QT = const.tile([64, TQ], dt_x)        # Q^T, local query cols
        wkv_sb = const.tile([128, 8, 128], dt_x)
        wq_sb = const.tile([128, 8, 64], dt_x)
        m32_sb = const.tile([128, 32], dt_x)
        ident = const.tile([64, 64], dt_x)
        zl = const.tile([1, 65], dt_x)         # zeros for PSUM-opening matmul
        zr = const.tile([1, 512], dt_x)

        xT_r = xT.rearrange("(a p) t -> p a t", p=128)  # [128, 8, T]

        # DMA issues first: weights/mask on gpsimd queue; x chunks in
        # reverse, halves split across the sync + scalar queues
        nc.gpsimd.dma_start(out=wkv_sb, in_=wkv)
        nc.gpsimd.dma_start(out=wq_sb, in_=wq)
        nc.gpsimd.dma_start(out=m32_sb, in_=m32)
        xts = {}
        for tci in range(7, -1, -1):
            ts = slice(tci * 512, (tci + 1) * 512)
            xt = xpool.tile([128, 8, 512], dt_x, tag="xt")
            nc.sync.dma_start(out=xt[:, 0:4, :], in_=xT_r[:, 0:4, ts])
            nc.scalar.dma_start(out=xt[:, 4:8, :], in_=xT_r[:, 4:8, ts])
            xts[tci] = xt

        nc.vector.memset(VS[:, :, 64:65], 1.0)
        nc.vector.memset(zl, 0.0)
        nc.vector.memset(zr, 0.0)
        make_identity(nc, ident)

        pv = psO.tile([65, TQ], f32)
        # open both accumulation half-banks with zeroing matmuls
        for h in range(2):
            nc.tensor.matmul(
                pv[:, h * 512:(h + 1) * 512],
                lhsT=zl, rhs=zr,
                start=True, stop=False, skip_group_check=True,
            )

        # ---- projection work, emitted as small closures ("pieces") so it
        # can be smeared between attention steps
        def proj_kv_pieces(tci):
            ts = slice(tci * 512, (tci + 1) * 512)
            xt = xts[tci]
            kv_ps = psP.tile([128, 512], f32, tag="pj")

            def mk(cc0):
                def f():
                    for cc in (cc0, cc0 + 1):
                        nc.tensor.matmul(
                            kv_ps,
                            lhsT=wkv_sb[:, cc, :],
                            rhs=xt[:, cc, :],
                            start=(cc == 0),
                            stop=(cc == 7),
                        )
                return f

            vt = stage.tile([64, 512], dt_x, tag="vt")

            def copies():
                nc.vector.tensor_copy(KT[0:64, ts], kv_ps[0:64, :])
                nc.vector.tensor_copy(vt, kv_ps[64:128, :])

            def vfix():
                vq = psP.tile([128, 4, 64], dt_x, tag="pj")
                for sub in range(4):
                    nc.tensor.matmul(
                        vq[:, sub, :],
                        lhsT=vt[:, sub * 128:(sub + 1) * 128],
                        rhs=ident,
                        is_transpose=True,
                        start=(sub == 0),
                        stop=(sub == 3),
                        skip_group_check=True,
                    )
                nc.vector.tensor_copy(VS[:, tci * 4:tci * 4 + 4, 0:64], vq)

            return [mk(0), mk(2), mk(4), mk(6), copies, vfix]

        def proj_q_pieces(tci):
            """Q projection for chunk tci -> QT columns [128tci, 128tci+128)."""
            qx = qxp.tile([128, 8, 128], dt_x, tag="qx")
            xt4 = xts[tci].rearrange("p a (f g) -> p a f g", g=4)

            def extract():
                nc.vector.tensor_copy(qx, xt4[:, :, :, 0])

            q_ps = psP.tile([64, 128], f32, tag="pj")

            def mk(cc0):
                def f():
                    for cc in (cc0, cc0 + 1):
                        nc.tensor.matmul(
                            q_ps,
                            lhsT=wq_sb[:, cc, :],
                            rhs=qx[:, cc, :],
                            start=(cc == 0),
                            stop=(cc == 7),
                        )
                return f

            def copy():
                nc.vector.tensor_copy(
                    QT[0:64, 128 * tci:128 * tci + 128], q_ps)

            return [extract, mk(0), mk(2), mk(4), mk(6), copy]

        # ---- attention
        def attend_S(kt, lo, hi):
            """S^T matmul for key tile kt on query columns [lo, hi)."""
            s_ps = psA.tile([128, 512], f32, tag="s")
            nc.tensor.matmul(
                s_ps[:, 0:hi - lo],
                lhsT=KT[:, kt * 128:(kt + 1) * 128],
                rhs=QT[:, lo:hi],
                start=True,
                stop=True,
            )
            return s_ps

        def attend_rest(kt, s_ps, lo, hi, masked, last):
            """exp -> band mask -> PV accumulate on query columns [lo, hi)."""
            w = hi - lo
            p_sb = ppool.tile([128, 512], dt_x, tag="p")
            nc.scalar.activation(
                p_sb[:, 0:w], s_ps[:, 0:w],
                mybir.ActivationFunctionType.Exp, scale=float(D) ** -0.5,
            )
            if masked:
                nc.vector.tensor_mul(p_sb[:, 0:32], p_sb[:, 0:32], m32_sb)
            nc.tensor.matmul(
                pv[:, lo:hi],
                lhsT=VS[:, kt, :],
                rhs=p_sb[:, 0:w],
                start=False,
                stop=last,
                skip_group_check=True,
            )

        # ---- schedule: phase 1 (columns >= 512 and upper suffixes), with
        # projection pieces smeared in; phase 2 (columns < 512) pure compute
        pending = []

        def queue_pieces(ps):
            pending.extend(ps)

        def drain(n):
            for _ in range(n):
                if pending:
                    pending.pop(0)()

        queue_pieces(proj_kv_pieces(7))
        queue_pieces(proj_q_pieces(7))
        drain(len(pending))          # chunk 7 projections up front
        queue_pieces(proj_kv_pieces(6))
        queue_pieces(proj_q_pieces(6))

        pipe = []  # [(kt, s_ps, lo, hi, masked)]

        def push(kt, lo, hi, masked):
            pipe.append((kt, attend_S(kt, lo, hi), lo, hi, masked))
            drain(2)
            if len(pipe) > 2:
                pkt, ps, plo, phi, pm = pipe.pop(0)
                attend_rest(pkt, ps, plo, phi, pm, last=False)

        for kt in range(NKT - 1, -1, -1):
            if kt % 4 == 3 and kt < NKT - 1:
                # entering chunk ch = kt//4: everything it needs must be
                # emitted now; then queue the next chunk's projections
                drain(len(pending))
                ch = kt // 4
                if ch >= 1:
                    queue_pieces(proj_kv_pieces(ch - 1))
                    queue_pieces(proj_q_pieces(ch - 1))
            lo = max(32 * kt, 512)
            push(kt, lo, TQ, masked=(kt >= 16))
        drain(len(pending))

        # drain phase-1 pipe: mark the last writer of pv[:, 512:]
        while pipe:
            pkt, ps, plo, phi, pm = pipe.pop(0)
            attend_rest(pkt, ps, plo, phi, pm, last=(not pipe))

        # output upper half overlaps phase 2
        osb = stage.tile([65, TQ], f32, tag="o")
        nc.vector.tensor_copy(osb[:, 512:], pv[:, 512:])
        nc.sync.dma_start(out=outT[:, 512:], in_=osb[:, 512:])

        for kt in range(15, -1, -1):
            push(kt, 32 * kt, 512, masked=True)
        while pipe:
            pkt, ps, plo, phi, pm = pipe.pop(0)
            attend_rest(pkt, ps, plo, phi, pm, last=(not pipe))

        nc.vector.tensor_copy(osb[:, 0:512], pv[:, 0:512])
        nc.sync.dma_start(out=outT[:, 0:512], in_=osb[:, 0:512])

    nc.compile()
    return nc


def _prep_inputs(x, Wq, Wk, Wv, np_dt):
    """Per-core input maps."""
    wkv = np.empty((128, 8, 128), dtype=np_dt)
    wkv[:, :, 0:64] = Wk.reshape(8, 128, 64).transpose(1, 0, 2)
    wkv[:, :, 64:128] = Wv.reshape(8, 128, 64).transpose(1, 0, 2)
    wq = np.ascontiguousarray(
        Wq.reshape(8, 128, 64).transpose(1, 0, 2)).astype(np_dt)

    s = np.arange(T)
    p_idx = np.arange(128)[:, None]
    col = np.arange(32)[None, :]

    in_maps = []
    for core in range(NCORES):
        b, c = divmod(core, 4)
        # column roll: slot s <- abs column 4*(s//4) + ((s%4 + c) % 4)
        perm = 4 * (s // 4) + ((s % 4 + c) % 4)
        xT = np.ascontiguousarray(x[b].T[:, perm]).astype(np_dt)
        # band mask: key slot p (within its tile) visible to band column col?
        abs_k = 4 * (p_idx // 4) + ((p_idx % 4 + c) % 4)
        abs_q = 4 * col + c
        m32 = (abs_k <= abs_q).astype(np_dt)
        in_maps.append({
            "xT": xT,
            "wkv": wkv,
            "wq": wq,
            "m32": m32,
        })
    return in_maps


def kernel(x, Wq, Wk, Wv, _trace=False, _trace_cores=None):
    from concourse.bass_utils import run_bass_kernel_spmd

    dt_x, np_dt = _dtypes()

    key = ("prog", str(dt_x))
    if key not in _CACHE:
        _CACHE[key] = _build_program(dt_x)
    nc = _CACHE[key]

    in_maps = _prep_inputs(
        np.asarray(x, np.float32), np.asarray(Wq, np.float32),
        np.asarray(Wk, np.float32), np.asarray(Wv, np.float32), np_dt)

    res = run_bass_kernel_spmd(
        nc, in_maps, core_ids=list(range(NCORES)), trace=_trace,
        trace_cores=_trace_cores)

    jidx = 4 * np.arange(TQ)
    out = np.empty((B, T, D), dtype=np.float32)
    for core in range(NCORES):
        b, c = divmod(core, 4)
        o = res.results[core]["outT"]  # [65, TQ]
        out[b, jidx + c, :] = (o[0:64, :] / o[64:65, :]).T
    if _trace:
        return out, res
    return out


# revision 7
# speedup vs baseline: 1.2909x; 1.2909x over previous
"""Causal single-head attention (B=2, T=4096, C=1024, D=64) on 8 TRN2 cores.

Sharding: core i -> batch b = i//4, query phase c = i%4: the core owns the
strided query rows {4j + c : j in [0,1024)}. This balances causal work
exactly across cores AND lets each core skip fully-masked key tiles:

  - x[b] is column-permuted on host (within every group of 4 columns,
    rotate by c) so the core's query columns sit at slots 4j — a
    compile-time stride-4 slice, identical on every core.
  - key tile kt (slots 128kt..128kt+127) is attended only by query
    columns j >= 32kt: columns j >= 32(kt+1) are fully visible,
    j in [32kt, 32kt+32) are the diagonal band (one host-computed
    [128,32] 0/1 mask, same for every kt), and j < 32kt are fully
    masked — never computed.
  - denominator comes free from a ones-column in V' (column 64); the
    kernel returns unnormalized [65, 1024] = [PV^T ; rowsum]; host divides.

Two attention phases over descending key tiles (chunks DMA'd in reverse;
chunk ch supplies key tiles 4ch..4ch+3 and query columns [128ch, 128ch+128)):
  phase 1: kt 31..0 on query columns [max(32kt,512), 1024) — only needs
           Q of chunks 4..7, S tiles are <=512 wide (1 PSUM bank), and
           pv[:, 512:] completes here so its output DMA overlaps phase 2.
  phase 2: kt 15..0 on query columns [32kt, 512) — pure compute tail.
Projection work is smeared between attends to keep PE dense; PV
accumulates into a PSUM bank pre-zeroed by a contraction-1 zero matmul.
"""

import numpy as np

B, T, C, D = 2, 4096, 1024, 64
NCORES = 8
TQ = 1024          # queries per core (strided by 4)
NKT = T // 128     # 32 key tiles of 128
DTYPE_NAME = "bfloat16"

_CACHE = {}


def _dtypes():
    import concourse.mybir as mybir
    if DTYPE_NAME == "bfloat16":
        import ml_dtypes
        return mybir.dt.bfloat16, ml_dtypes.bfloat16
    return mybir.dt.float32, np.float32


def _build_program(dt_x):
    import concourse.bass as bass
    import concourse.mybir as mybir
    import concourse.tile as tile
    from concourse import bacc
    from concourse.masks import make_identity
    from contextlib import ExitStack

    f32 = mybir.dt.float32

    nc = bacc.Bacc(
        "TRN2",
        target_bir_lowering=False,
        debug=False,
        num_devices=NCORES,
    )

    xT_t = nc.dram_tensor("xT", [C, T], dt_x, kind="ExternalInput")
    wkv_t = nc.dram_tensor("wkv", [128, 8, 128], dt_x, kind="ExternalInput")
    wq_t = nc.dram_tensor("wq", [128, 8, 64], dt_x, kind="ExternalInput")
    m32_t = nc.dram_tensor("m32", [128, 32], dt_x, kind="ExternalInput")
    out_t = nc.dram_tensor("outT", [65, TQ], f32, kind="ExternalOutput")

    xT = xT_t.ap()
    wkv = wkv_t.ap()
    wq = wq_t.ap()
    m32 = m32_t.ap()
    outT = out_t.ap()

    with tile.TileContext(nc) as tc, ExitStack() as ctx:
        const = ctx.enter_context(tc.tile_pool(name="const", bufs=1))
        xpool = ctx.enter_context(tc.tile_pool(name="xpool", bufs=8))
        stage = ctx.enter_context(tc.tile_pool(name="stage", bufs=3))
        qxp = ctx.enter_context(tc.tile_pool(name="qxp", bufs=2))
        ppool = ctx.enter_context(tc.tile_pool(name="ppool", bufs=4))
        psA = ctx.enter_context(tc.tile_pool(name="psA", bufs=3, space="PSUM"))
        psP = ctx.enter_context(tc.tile_pool(name="psP", bufs=2, space="PSUM"))
        psO = ctx.enter_context(tc.tile_pool(name="psO", bufs=1, space="PSUM"))

        # persistent SBUF tensors
        KT = const.tile([64, T], dt_x)         # K^T, key slot order
        VS = const.tile([128, NKT, 65], dt_x)  # V': [:, kt, 0:64] = V, col 64 = 1
        QT = const.tile([64, TQ], dt_x)        # Q^T, local query cols
        wkv_sb = const.tile([128, 8, 128], dt_x)
        wq_sb = const.tile([128, 8, 64], dt_x)
        m32_sb = const.tile([128, 32], dt_x)
        ident = const.tile([64, 64], dt_x)
        zl = const.tile([1, 65], dt_x)         # zeros for PSUM-opening matmul
        zr = const.tile([1, 512], dt_x)

        xT_r = xT.rearrange("(a p) t -> p a t", p=128)  # [128, 8, T]

        # DMA issues first: weights/mask on the scalar queue (3 small
        # transfers, done before the first exp); all x chunks on the sync
        # queue in reverse order, chunk 7 split in halves so its first
        # KV matmuls can start half a chunk earlier
        nc.scalar.dma_start(out=wkv_sb, in_=wkv)
        nc.scalar.dma_start(out=wq_sb, in_=wq)
        nc.scalar.dma_start(out=m32_sb, in_=m32)
        xts = {}
        for tci in range(7, -1, -1):
            ts = slice(tci * 512, (tci + 1) * 512)
            xt = xpool.tile([128, 8, 512], dt_x, tag="xt")
            if tci == 7:
                nc.sync.dma_start(out=xt[:, 0:4, :], in_=xT_r[:, 0:4, ts])
                nc.sync.dma_start(out=xt[:, 4:8, :], in_=xT_r[:, 4:8, ts])
            else:
                nc.sync.dma_start(out=xt, in_=xT_r[:, :, ts])
            xts[tci] = xt

        nc.vector.memset(VS[:, :, 64:65], 1.0)
        nc.vector.memset(zl, 0.0)
        nc.vector.memset(zr, 0.0)
        make_identity(nc, ident)

        pv = psO.tile([65, TQ], f32)
        # open both accumulation half-banks with zeroing matmuls
        for h in range(2):
            nc.tensor.matmul(
                pv[:, h * 512:(h + 1) * 512],
                lhsT=zl, rhs=zr,
                start=True, stop=False, skip_group_check=True,
            )

        # ---- projection work, emitted as small closures ("pieces") so it
        # can be smeared between attention steps
        def proj_kv_pieces(tci):
            ts = slice(tci * 512, (tci + 1) * 512)
            xt = xts[tci]
            kv_ps = psP.tile([128, 512], f32, tag="pj")

            def mk(cc0):
                def f():
                    for cc in (cc0, cc0 + 1):
                        nc.tensor.matmul(
                            kv_ps,
                            lhsT=wkv_sb[:, cc, :],
                            rhs=xt[:, cc, :],
                            start=(cc == 0),
                            stop=(cc == 7),
                        )
                return f

            vt = stage.tile([64, 512], dt_x, tag="vt")

            def copies():
                nc.vector.tensor_copy(KT[0:64, ts], kv_ps[0:64, :])
                nc.vector.tensor_copy(vt, kv_ps[64:128, :])

            def vfix():
                vq = psP.tile([128, 4, 64], dt_x, tag="pj")
                for sub in range(4):
                    nc.tensor.matmul(
                        vq[:, sub, :],
                        lhsT=vt[:, sub * 128:(sub + 1) * 128],
                        rhs=ident,
                        is_transpose=True,
                        start=(sub == 0),
                        stop=(sub == 3),
                        skip_group_check=True,
                    )
                nc.vector.tensor_copy(VS[:, tci * 4:tci * 4 + 4, 0:64], vq)

            return [mk(0), mk(2), mk(4), mk(6), copies, vfix]

        def proj_q_pieces(tci):
            """Q projection for chunk tci -> QT columns [128tci, 128tci+128)."""
            qx = qxp.tile([128, 8, 128], dt_x, tag="qx")
            xt4 = xts[tci].rearrange("p a (f g) -> p a f g", g=4)

            def extract():
                nc.vector.tensor_copy(qx, xt4[:, :, :, 0])

            q_ps = psP.tile([64, 128], f32, tag="pj")

            def mk(cc0):
                def f():
                    for cc in (cc0, cc0 + 1):
                        nc.tensor.matmul(
                            q_ps,
                            lhsT=wq_sb[:, cc, :],
                            rhs=qx[:, cc, :],
                            start=(cc == 0),
                            stop=(cc == 7),
                        )
                return f

            def copy():
                nc.vector.tensor_copy(
                    QT[0:64, 128 * tci:128 * tci + 128], q_ps)

            return [extract, mk(0), mk(2), mk(4), mk(6), copy]

        # ---- attention
        def attend_S(kt, lo, hi):
            """S^T matmul for key tile kt on query columns [lo, hi)."""
            s_ps = psA.tile([128, 512], f32, tag="s")
            nc.tensor.matmul(
                s_ps[:, 0:hi - lo],
                lhsT=KT[:, kt * 128:(kt + 1) * 128],
                rhs=QT[:, lo:hi],
                start=True,
                stop=True,
            )
            return s_ps

        def attend_rest(kt, s_ps, lo, hi, masked, last):
            """exp -> band mask -> PV accumulate on query columns [lo, hi)."""
            w = hi - lo
            p_sb = ppool.tile([128, 512], dt_x, tag="p")
            nc.scalar.activation(
                p_sb[:, 0:w], s_ps[:, 0:w],
                mybir.ActivationFunctionType.Exp, scale=float(D) ** -0.5,
            )
            if masked:
                nc.vector.tensor_mul(p_sb[:, 0:32], p_sb[:, 0:32], m32_sb)
            nc.tensor.matmul(
                pv[:, lo:hi],
                lhsT=VS[:, kt, :],
                rhs=p_sb[:, 0:w],
                start=False,
                stop=last,
                skip_group_check=True,
            )

        # ---- schedule: phase 1 (columns >= 512 and upper suffixes), with
        # projection pieces smeared in; phase 2 (columns < 512) pure compute
        pending = []

        def queue_pieces(ps):
            pending.extend(ps)

        def drain(n):
            for _ in range(n):
                if pending:
                    pending.pop(0)()

        queue_pieces(proj_kv_pieces(7))
        queue_pieces(proj_q_pieces(7))
        drain(len(pending))          # chunk 7 projections up front
        queue_pieces(proj_kv_pieces(6))
        queue_pieces(proj_q_pieces(6))

        pipe = []  # [(kt, s_ps, lo, hi, masked)]

        def push(kt, lo, hi, masked):
            pipe.append((kt, attend_S(kt, lo, hi), lo, hi, masked))
            drain(2)
            if len(pipe) > 2:
                pkt, ps, plo, phi, pm = pipe.pop(0)
                attend_rest(pkt, ps, plo, phi, pm, last=False)

        for kt in range(NKT - 1, -1, -1):
            if kt % 4 == 3 and kt < NKT - 1:
                # entering chunk ch = kt//4: everything it needs must be
                # emitted now; then queue the next chunk's projections
                drain(len(pending))
                ch = kt // 4
                if ch >= 1:
                    queue_pieces(proj_kv_pieces(ch - 1))
                    queue_pieces(proj_q_pieces(ch - 1))
            lo = max(32 * kt, 512)
            push(kt, lo, TQ, masked=(kt >= 16))
        drain(len(pending))

        # drain phase-1 pipe: mark the last writer of pv[:, 512:]
        while pipe:
            pkt, ps, plo, phi, pm = pipe.pop(0)
            attend_rest(pkt, ps, plo, phi, pm, last=(not pipe))

        # output upper half overlaps phase 2
        osb = stage.tile([65, TQ], f32, tag="o")
        nc.vector.tensor_copy(osb[:, 512:], pv[:, 512:])
        nc.sync.dma_start(out=outT[:, 512:], in_=osb[:, 512:])

        for kt in range(15, -1, -1):
            push(kt, 32 * kt, 512, masked=True)
        while pipe:
            pkt, ps, plo, phi, pm = pipe.pop(0)
            attend_rest(pkt, ps, plo, phi, pm, last=(not pipe))

        nc.vector.tensor_copy(osb[:, 0:512], pv[:, 0:512])
        nc.sync.dma_start(out=outT[:, 0:512], in_=osb[:, 0:512])

    nc.compile()
    return nc


def _prep_inputs(x, Wq, Wk, Wv, np_dt):
    """Per-core input maps."""
    wkv = np.empty((128, 8, 128), dtype=np_dt)
    wkv[:, :, 0:64] = Wk.reshape(8, 128, 64).transpose(1, 0, 2)
    wkv[:, :, 64:128] = Wv.reshape(8, 128, 64).transpose(1, 0, 2)
    wq = np.ascontiguousarray(
        Wq.reshape(8, 128, 64).transpose(1, 0, 2)).astype(np_dt)

    s = np.arange(T)
    p_idx = np.arange(128)[:, None]
    col = np.arange(32)[None, :]

    in_maps = []
    for core in range(NCORES):
        b, c = divmod(core, 4)
        # column roll: slot s <- abs column 4*(s//4) + ((s%4 + c) % 4)
        perm = 4 * (s // 4) + ((s % 4 + c) % 4)
        xT = np.ascontiguousarray(x[b].T[:, perm]).astype(np_dt)
        # band mask: key slot p (within its tile) visible to band column col?
        abs_k = 4 * (p_idx // 4) + ((p_idx % 4 + c) % 4)
        abs_q = 4 * col + c
        m32 = (abs_k <= abs_q).astype(np_dt)
        in_maps.append({
            "xT": xT,
            "wkv": wkv,
            "wq": wq,
            "m32": m32,
        })
    return in_maps


def kernel(x, Wq, Wk, Wv, _trace=False, _trace_cores=None):
    from concourse.bass_utils import run_bass_kernel_spmd

    dt_x, np_dt = _dtypes()

    key = ("prog", str(dt_x))
    if key not in _CACHE:
        _CACHE[key] = _build_program(dt_x)
    nc = _CACHE[key]

    in_maps = _prep_inputs(
        np.asarray(x, np.float32), np.asarray(Wq, np.float32),
        np.asarray(Wk, np.float32), np.asarray(Wv, np.float32), np_dt)

    res = run_bass_kernel_spmd(
        nc, in_maps, core_ids=list(range(NCORES)), trace=_trace,
        trace_cores=_trace_cores)

    jidx = 4 * np.arange(TQ)
    out = np.empty((B, T, D), dtype=np.float32)
    for core in range(NCORES):
        b, c = divmod(core, 4)
        o = res.results[core]["outT"]  # [65, TQ]
        out[b, jidx + c, :] = (o[0:64, :] / o[64:65, :]).T
    if _trace:
        return out, res
    return out


# revision 20
# speedup vs baseline: 1.3077x; 1.0130x over previous
"""Causal single-head attention (B=2, T=4096, C=1024, D=64) on 8 TRN2 cores.

Sharding: core i -> batch b = i//4, query phase c = i%4: the core owns the
strided query rows {4j + c : j in [0,1024)}. This balances causal work
exactly across cores AND lets each core skip fully-masked key tiles:

  - x[b] is column-permuted on host (within every group of 4 columns,
    rotate by c) so the core's query columns sit at slots 4j — a
    compile-time stride-4 slice, identical on every core.
  - key tile kt (slots 128kt..128kt+127) is attended only by query
    columns j >= 32kt: columns j >= 32(kt+1) are fully visible,
    j in [32kt, 32kt+32) are the diagonal band (one host-computed
    [128,32] 0/1 mask, same for every kt), and j < 32kt are fully
    masked — never computed.
  - denominator comes free from a ones-column in V' (column 64); the
    kernel returns unnormalized [65, 1024] = [PV^T ; rowsum]; host divides.

Key tiles are processed in DESCENDING order so attention starts as soon
as the LAST x chunk arrives (chunks DMA'd in reverse): chunk ch supplies
both key tiles 4ch..4ch+3 and query columns [128ch, 128(ch+1)), and key
tile kt only needs query columns [32kt, 1024) — exactly what's loaded.
Each chunk's projections are emitted right after the first attend of the
previous chunk so the PE chews projection matmuls while DMA streams; PV
accumulates into PSUM half-banks opened by a contraction-1 zero matmul.
"""

import numpy as np

B, T, C, D = 2, 4096, 1024, 64
NCORES = 8
TQ = 1024          # queries per core (strided by 4)
NKT = T // 128     # 32 key tiles of 128
DTYPE_NAME = "bfloat16"

_CACHE = {}


def _dtypes():
    import concourse.mybir as mybir
    if DTYPE_NAME == "bfloat16":
        import ml_dtypes
        return mybir.dt.bfloat16, ml_dtypes.bfloat16
    return mybir.dt.float32, np.float32


def _build_program(dt_x):
    import concourse.bass as bass
    import concourse.mybir as mybir
    import concourse.tile as tile
    from concourse import bacc
    from concourse.masks import make_identity
    from contextlib import ExitStack

    f32 = mybir.dt.float32

    nc = bacc.Bacc(
        "TRN2",
        target_bir_lowering=False,
        debug=False,
        num_devices=NCORES,
    )

    xT_t = nc.dram_tensor("xT", [C, T], dt_x, kind="ExternalInput")
    wkv_t = nc.dram_tensor("wkv", [128, 8, 128], dt_x, kind="ExternalInput")
    wq_t = nc.dram_tensor("wq", [128, 8, 64], dt_x, kind="ExternalInput")
    m32_t = nc.dram_tensor("m32", [128, 32], dt_x, kind="ExternalInput")
    out_t = nc.dram_tensor("outT", [65, TQ], f32, kind="ExternalOutput")

    xT = xT_t.ap()
    wkv = wkv_t.ap()
    wq = wq_t.ap()
    m32 = m32_t.ap()
    outT = out_t.ap()

    with tile.TileContext(nc) as tc, ExitStack() as ctx:
        const = ctx.enter_context(tc.tile_pool(name="const", bufs=1))
        xpool = ctx.enter_context(tc.tile_pool(name="xpool", bufs=8))
        stage = ctx.enter_context(tc.tile_pool(name="stage", bufs=3))
        qxp = ctx.enter_context(tc.tile_pool(name="qxp", bufs=2))
        ppool = ctx.enter_context(tc.tile_pool(name="ppool", bufs=3))
        psA = ctx.enter_context(tc.tile_pool(name="psA", bufs=2, space="PSUM"))
        psP = ctx.enter_context(tc.tile_pool(name="psP", bufs=2, space="PSUM"))
        psO = ctx.enter_context(tc.tile_pool(name="psO", bufs=1, space="PSUM"))

        # persistent SBUF tensors
        KT = const.tile([64, T], dt_x)         # K^T, key slot order
        VS = const.tile([128, NKT, 65], dt_x)  # V': [:, kt, 0:64] = V, col 64 = 1
        QT = const.tile([64, TQ], dt_x)        # Q^T, local query cols
        wkv_sb = const.tile([128, 8, 128], dt_x)
        wq_sb = const.tile([128, 8, 64], dt_x)
        m32_sb = const.tile([128, 32], dt_x)
        ident = const.tile([64, 64], dt_x)
        zl = const.tile([1, 65], dt_x)         # zeros for PSUM-opening matmul
        zr = const.tile([1, 512], dt_x)

        xT_r = xT.rearrange("(a p) t -> p a t", p=128)  # [128, 8, T]

        # DMA issues first: weights/mask on the scalar queue (3 small
        # transfers, done before the first exp); all x chunks on the sync
        # queue in reverse order, chunk 7 split in halves so its first
        # KV matmuls can start half a chunk earlier
        nc.scalar.dma_start(out=wkv_sb, in_=wkv)
        nc.scalar.dma_start(out=wq_sb, in_=wq)
        nc.scalar.dma_start(out=m32_sb, in_=m32)
        xts = {}
        for tci in range(7, -1, -1):
            ts = slice(tci * 512, (tci + 1) * 512)
            xt = xpool.tile([128, 8, 512], dt_x, tag="xt")
            nc.sync.dma_start(out=xt, in_=xT_r[:, :, ts])
            xts[tci] = xt

        nc.vector.memset(VS[:, :, 64:65], 1.0)
        nc.vector.memset(zl, 0.0)
        nc.vector.memset(zr, 0.0)
        make_identity(nc, ident)

        pv = psO.tile([65, TQ], f32)
        # open both pv accumulation half-banks with zeroing matmuls
        for h in range(2):
            nc.tensor.matmul(
                pv[:, h * 512:(h + 1) * 512],
                lhsT=zl, rhs=zr,
                start=True, stop=False, skip_group_check=True,
            )

        # ---- projection work, emitted as small closures ("pieces") so it
        # can be smeared between attention steps
        def proj_kv_pieces(tci):
            ts = slice(tci * 512, (tci + 1) * 512)
            xt = xts[tci]
            kv_ps = psP.tile([128, 512], f32, tag="pj")

            def mk(cc0):
                def f():
                    for cc in (cc0, cc0 + 1):
                        nc.tensor.matmul(
                            kv_ps,
                            lhsT=wkv_sb[:, cc, :],
                            rhs=xt[:, cc, :],
                            start=(cc == 0),
                            stop=(cc == 7),
                        )
                return f

            vt = stage.tile([64, 512], dt_x, tag="vt")

            def copies():
                nc.vector.tensor_copy(KT[0:64, ts], kv_ps[0:64, :])
                nc.vector.tensor_copy(vt, kv_ps[64:128, :])

            def vfix():
                # V^T chunk -> VS tiles (4 PE transposes packed in one bank)
                vq = psP.tile([128, 4, 64], dt_x, tag="pj")
                for sub in range(4):
                    nc.tensor.matmul(
                        vq[:, sub, :],
                        lhsT=vt[:, sub * 128:(sub + 1) * 128],
                        rhs=ident,
                        is_transpose=True,
                        start=(sub == 0),
                        stop=(sub == 3),
                        skip_group_check=True,
                    )
                nc.vector.tensor_copy(VS[:, tci * 4:tci * 4 + 4, 0:64], vq)

            return [mk(0), mk(2), mk(4), mk(6), copies, vfix]

        def proj_q_pieces(tci):
            """Q projection for chunk tci -> QT columns [128tci, 128tci+128)."""
            qx = qxp.tile([128, 8, 128], dt_x, tag="qx")
            xt4 = xts[tci].rearrange("p a (f g) -> p a f g", g=4)

            def extract():
                nc.vector.tensor_copy(qx, xt4[:, :, :, 0])

            q_ps = psP.tile([64, 128], f32, tag="pj")

            def mk(cc0):
                def f():
                    for cc in (cc0, cc0 + 1):
                        nc.tensor.matmul(
                            q_ps,
                            lhsT=wq_sb[:, cc, :],
                            rhs=qx[:, cc, :],
                            start=(cc == 0),
                            stop=(cc == 7),
                        )
                return f

            def copy():
                nc.vector.tensor_copy(
                    QT[0:64, 128 * tci:128 * tci + 128], q_ps)

            return [extract, mk(0), mk(2), mk(4), mk(6), copy]

        # ---- attention
        def spans_of(kt):
            qlo = 32 * kt
            return [(qlo, 512), (512, TQ)] if qlo < 512 else [(qlo, TQ)]

        def attend_S(kt):
            s_ps = psA.tile([128, TQ], f32, tag="s")
            for lo, hi in spans_of(kt):
                nc.tensor.matmul(
                    s_ps[:, lo:hi],
                    lhsT=KT[:, kt * 128:(kt + 1) * 128],
                    rhs=QT[:, lo:hi],
                    start=True,
                    stop=True,
                )
            return s_ps

        def attend_rest(kt, s_ps, last):
            qlo = 32 * kt
            p_sb = ppool.tile([128, TQ], dt_x, tag="p")
            nc.scalar.activation(
                p_sb[:, qlo:], s_ps[:, qlo:],
                mybir.ActivationFunctionType.Exp, scale=float(D) ** -0.5,
            )
            nc.vector.tensor_mul(
                p_sb[:, qlo:qlo + 32], p_sb[:, qlo:qlo + 32], m32_sb)
            for lo, hi in spans_of(kt):
                nc.tensor.matmul(
                    pv[:, lo:hi],
                    lhsT=VS[:, kt, :],
                    rhs=p_sb[:, lo:hi],
                    start=False,
                    stop=last,
                    skip_group_check=True,
                )

        # ---- schedule: descending key tiles; next chunk's projection
        # pieces smeared between attends
        pending = []

        def queue_pieces(ps):
            pending.extend(ps)

        def drain(n):
            for _ in range(n):
                if pending:
                    pending.pop(0)()

        def proj_chunk(tci):
            for f in proj_kv_pieces(tci) + proj_q_pieces(tci):
                f()

        proj_chunk(7)
        pipe = []  # [(kt, s_ps)]
        for kt in range(NKT - 1, -1, -1):
            pipe.append((kt, attend_S(kt)))
            if kt % 4 == 3 and kt >= 4:
                proj_chunk(kt // 4 - 1)
            if len(pipe) > 1:
                pkt, ps = pipe.pop(0)
                attend_rest(pkt, ps, last=False)
        pkt, ps = pipe.pop(0)
        attend_rest(pkt, ps, last=True)

        osb = stage.tile([65, TQ], f32, tag="o")
        for qh in range(2):  # halves so copy/DMA overlap the last PV matmul
            qs = slice(qh * 512, (qh + 1) * 512)
            nc.vector.tensor_copy(osb[:, qs], pv[:, qs])
            nc.sync.dma_start(out=outT[:, qs], in_=osb[:, qs])

    nc.compile()
    return nc


def _prep_inputs(x, Wq, Wk, Wv, np_dt):
    """Per-core input maps."""
    wkv = np.empty((128, 8, 128), dtype=np_dt)
    wkv[:, :, 0:64] = Wk.reshape(8, 128, 64).transpose(1, 0, 2)
    wkv[:, :, 64:128] = Wv.reshape(8, 128, 64).transpose(1, 0, 2)
    wq = np.ascontiguousarray(
        Wq.reshape(8, 128, 64).transpose(1, 0, 2)).astype(np_dt)

    s = np.arange(T)
    p_idx = np.arange(128)[:, None]
    col = np.arange(32)[None, :]

    in_maps = []
    for core in range(NCORES):
        b, c = divmod(core, 4)
        # column roll: slot s <- abs column 4*(s//4) + ((s%4 + c) % 4)
        perm = 4 * (s // 4) + ((s % 4 + c) % 4)
        xT = np.ascontiguousarray(x[b].T[:, perm]).astype(np_dt)
        # band mask: key slot p (within its tile) visible to band column col?
        abs_k = 4 * (p_idx // 4) + ((p_idx % 4 + c) % 4)
        abs_q = 4 * col + c
        m32 = (abs_k <= abs_q).astype(np_dt)
        in_maps.append({
            "xT": xT,
            "wkv": wkv,
            "wq": wq,
            "m32": m32,
        })
    return in_maps


def kernel(x, Wq, Wk, Wv, _trace=False, _trace_cores=None):
    from concourse.bass_utils import run_bass_kernel_spmd

    dt_x, np_dt = _dtypes()

    key = ("prog", str(dt_x))
    if key not in _CACHE:
        _CACHE[key] = _build_program(dt_x)
    nc = _CACHE[key]

    in_maps = _prep_inputs(
        np.asarray(x, np.float32), np.asarray(Wq, np.float32),
        np.asarray(Wk, np.float32), np.asarray(Wv, np.float32), np_dt)

    res = run_bass_kernel_spmd(
        nc, in_maps, core_ids=list(range(NCORES)), trace=_trace,
        trace_cores=_trace_cores)

    jidx = 4 * np.arange(TQ)
    out = np.empty((B, T, D), dtype=np.float32)
    for core in range(NCORES):
        b, c = divmod(core, 4)
        o = res.results[core]["outT"]  # [65, TQ]
        out[b, jidx + c, :] = (o[0:64, :] / o[64:65, :]).T
    if _trace:
        return out, res
    return out


# revision 23
# speedup vs baseline: 1.4195x; 1.0855x over previous
"""Causal single-head attention (B=2, T=4096, C=1024, D=64) on 8 TRN2 cores.

Sharding: core i -> batch b = i//4, query phase c = i%4: the core owns the
strided query rows {4j + c : j in [0,1024)}. This balances causal work
exactly across cores AND lets each core skip fully-masked key tiles:

  - x[b] is column-permuted on host (within every group of 4 columns,
    rotate by c) so the core's query columns sit at slots 4j — a
    compile-time stride-4 slice, identical on every core.
  - key tile kt (slots 128kt..128kt+127) is attended only by query
    columns j >= 32kt: columns j >= 32(kt+1) are fully visible,
    j in [32kt, 32kt+32) are the diagonal band (one host-computed
    [128,32] 0/1 mask, same for every kt), and j < 32kt are fully
    masked — never computed.
  - denominator comes free from a ones-column in V' (column 64); the
    kernel returns unnormalized [65, 1024] = [PV^T ; rowsum]; host divides.

Key tiles are processed in DESCENDING order so attention starts as soon
as the LAST x chunk arrives (chunks DMA'd in reverse): chunk ch supplies
both key tiles 4ch..4ch+3 and query columns [128ch, 128(ch+1)), and key
tile kt only needs query columns [32kt, 1024) — exactly what's loaded.
Each chunk's projections are emitted right after the first attend of the
previous chunk so the PE chews projection matmuls while DMA streams; PV
accumulates into PSUM half-banks opened by a contraction-1 zero matmul.
"""

import numpy as np

B, T, C, D = 2, 4096, 1024, 64
NCORES = 8
TQ = 1024          # queries per core (strided by 4)
NKT = T // 128     # 32 key tiles of 128
DTYPE_NAME = "bfloat16"

_CACHE = {}


def _dtypes():
    import concourse.mybir as mybir
    if DTYPE_NAME == "bfloat16":
        import ml_dtypes
        return mybir.dt.bfloat16, ml_dtypes.bfloat16
    return mybir.dt.float32, np.float32


def _build_program(dt_x):
    import concourse.bass as bass
    import concourse.mybir as mybir
    import concourse.tile as tile
    from concourse import bacc
    from concourse.masks import make_identity
    from contextlib import ExitStack

    f32 = mybir.dt.float32

    nc = bacc.Bacc(
        "TRN2",
        target_bir_lowering=False,
        debug=False,
        num_devices=NCORES,
    )

    xT_t = nc.dram_tensor("xT", [C, T], dt_x, kind="ExternalInput")
    wkv_t = nc.dram_tensor("wkv", [128, 8, 128], dt_x, kind="ExternalInput")
    wq_t = nc.dram_tensor("wq", [128, 8, 64], dt_x, kind="ExternalInput")
    m32_t = nc.dram_tensor("m32", [128, 32], dt_x, kind="ExternalInput")
    out_t = nc.dram_tensor("outT", [65, TQ], f32, kind="ExternalOutput")

    xT = xT_t.ap()
    wkv = wkv_t.ap()
    wq = wq_t.ap()
    m32 = m32_t.ap()
    outT = out_t.ap()

    with tile.TileContext(nc) as tc, ExitStack() as ctx:
        const = ctx.enter_context(tc.tile_pool(name="const", bufs=1))
        xpool = ctx.enter_context(tc.tile_pool(name="xpool", bufs=8))
        stage = ctx.enter_context(tc.tile_pool(name="stage", bufs=3))
        qxp = ctx.enter_context(tc.tile_pool(name="qxp", bufs=2))
        ppool = ctx.enter_context(tc.tile_pool(name="ppool", bufs=3))
        psA = ctx.enter_context(tc.tile_pool(name="psA", bufs=2, space="PSUM"))
        psP = ctx.enter_context(tc.tile_pool(name="psP", bufs=2, space="PSUM"))
        psO = ctx.enter_context(tc.tile_pool(name="psO", bufs=1, space="PSUM"))

        # persistent SBUF tensors
        KT = const.tile([64, T], dt_x)         # K^T, key slot order
        VS = const.tile([128, NKT, 65], dt_x)  # V': [:, kt, 0:64] = V, col 64 = 1
        QT = const.tile([64, TQ], dt_x)        # Q^T, local query cols
        wkv_sb = const.tile([128, 8, 128], dt_x)
        wq_sb = const.tile([128, 8, 64], dt_x)
        m32_sb = const.tile([128, 32], dt_x)
        ident = const.tile([64, 64], dt_x)
        zl = const.tile([1, 65], dt_x)         # zeros for PSUM-opening matmul
        zr = const.tile([1, 512], dt_x)

        xT_r = xT.rearrange("(a p) t -> p a t", p=128)  # [128, 8, T]

        # DMA issues first: weights/mask on the scalar queue (3 small
        # transfers, done before the first exp); all x chunks on the sync
        # queue in reverse order (attention consumes key tiles descending)
        nc.scalar.dma_start(out=wkv_sb, in_=wkv)
        nc.scalar.dma_start(out=wq_sb, in_=wq)
        nc.scalar.dma_start(out=m32_sb, in_=m32)
        xts = {}
        for tci in range(7, -1, -1):
            ts = slice(tci * 512, (tci + 1) * 512)
            xt = xpool.tile([128, 8, 512], dt_x, tag="xt")
            nc.sync.dma_start(out=xt[:, 0:4, :], in_=xT_r[:, 0:4, ts])
            nc.sync.dma_start(out=xt[:, 4:8, :], in_=xT_r[:, 4:8, ts])
            xts[tci] = xt

        nc.vector.memset(VS[:, :, 64:65], 1.0)
        nc.vector.memset(zl, 0.0)
        nc.vector.memset(zr, 0.0)
        make_identity(nc, ident)

        pv = psO.tile([65, TQ], f32)
        # open both pv accumulation half-banks with zeroing matmuls
        for h in range(2):
            nc.tensor.matmul(
                pv[:, h * 512:(h + 1) * 512],
                lhsT=zl, rhs=zr,
                start=True, stop=False, skip_group_check=True,
            )

        # ---- projection work for one chunk, as a list of closures
        def proj_kv_pieces(tci):
            ts = slice(tci * 512, (tci + 1) * 512)
            xt = xts[tci]
            kv_ps = psP.tile([128, 512], f32, tag="pj")

            def mk(cc0):
                def f():
                    for cc in (cc0, cc0 + 1):
                        nc.tensor.matmul(
                            kv_ps,
                            lhsT=wkv_sb[:, cc, :],
                            rhs=xt[:, cc, :],
                            start=(cc == 0),
                            stop=(cc == 7),
                        )
                return f

            vt = stage.tile([64, 512], dt_x, tag="vt")

            def copies():
                nc.vector.tensor_copy(KT[0:64, ts], kv_ps[0:64, :])
                nc.vector.tensor_copy(vt, kv_ps[64:128, :])

            def vfix():
                # V^T chunk -> VS tiles (4 PE transposes packed in one bank)
                vq = psP.tile([128, 4, 64], dt_x, tag="pj")
                for sub in range(4):
                    nc.tensor.matmul(
                        vq[:, sub, :],
                        lhsT=vt[:, sub * 128:(sub + 1) * 128],
                        rhs=ident,
                        is_transpose=True,
                        start=(sub == 0),
                        stop=(sub == 3),
                        skip_group_check=True,
                    )
                nc.vector.tensor_copy(VS[:, tci * 4:tci * 4 + 4, 0:64], vq)

            return [mk(0), mk(2), mk(4), mk(6), copies, vfix]

        def proj_q_pieces(tci):
            """Q projection for chunk tci -> QT columns [128tci, 128tci+128)."""
            qx = qxp.tile([128, 8, 128], dt_x, tag="qx")
            xt4 = xts[tci].rearrange("p a (f g) -> p a f g", g=4)

            def extract():
                nc.vector.tensor_copy(qx, xt4[:, :, :, 0])

            q_ps = psP.tile([64, 128], f32, tag="pj")

            def mk(cc0):
                def f():
                    for cc in (cc0, cc0 + 1):
                        nc.tensor.matmul(
                            q_ps,
                            lhsT=wq_sb[:, cc, :],
                            rhs=qx[:, cc, :],
                            start=(cc == 0),
                            stop=(cc == 7),
                        )
                return f

            def copy():
                nc.vector.tensor_copy(
                    QT[0:64, 128 * tci:128 * tci + 128], q_ps)

            return [extract, mk(0), mk(2), mk(4), mk(6), copy]

        # ---- attention
        def spans_of(kt):
            qlo = 32 * kt
            return [(qlo, 512), (512, TQ)] if qlo < 512 else [(qlo, TQ)]

        def attend_S(kt):
            s_ps = psA.tile([128, TQ], f32, tag="s")
            for lo, hi in spans_of(kt):
                nc.tensor.matmul(
                    s_ps[:, lo:hi],
                    lhsT=KT[:, kt * 128:(kt + 1) * 128],
                    rhs=QT[:, lo:hi],
                    start=True,
                    stop=True,
                )
            return s_ps

        def attend_rest(kt, s_ps, last, between=None):
            qlo = 32 * kt
            p_sb = ppool.tile([128, TQ], dt_x, tag="p")
            nc.scalar.activation(
                p_sb[:, qlo:], s_ps[:, qlo:],
                mybir.ActivationFunctionType.Exp, scale=float(D) ** -0.5,
            )
            nc.vector.tensor_mul(
                p_sb[:, qlo:qlo + 32], p_sb[:, qlo:qlo + 32], m32_sb)
            spans = spans_of(kt)
            if between is not None:
                spans = spans[::-1]  # upper half first, output it, then lower
            for i, (lo, hi) in enumerate(spans):
                nc.tensor.matmul(
                    pv[:, lo:hi],
                    lhsT=VS[:, kt, :],
                    rhs=p_sb[:, lo:hi],
                    start=False,
                    stop=last,
                    skip_group_check=True,
                )
                if between is not None and i == 0:
                    between()

        # ---- schedule: descending key tiles; the next chunk's projection
        # matmuls are emitted right after the first attend of each chunk so
        # the PE has projection work while ACT runs exp
        vfixes = {}

        def proj_chunk(tci):
            kv = proj_kv_pieces(tci)
            q = proj_q_pieces(tci)
            q[0]()                 # qx extract first: DVE does it while
            for f in kv[:-1]:      # the PE runs this chunk's KV matmuls
                f()
            vfixes[tci] = kv[-1]   # vfix deferred: runs right before the
            for f in q[1:]:        # chunk's own attends, when its vt
                f()                # copy has long completed

        proj_chunk(7)
        pipe = []  # [(kt, s_ps)]
        for kt in range(NKT - 1, -1, -1):
            pipe.append((kt, attend_S(kt)))
            if kt % 4 == 3:
                if kt // 4 in vfixes:
                    vfixes.pop(kt // 4)()
                if kt >= 4:
                    proj_chunk(kt // 4 - 1)
            if len(pipe) > 1:
                pkt, ps = pipe.pop(0)
                attend_rest(pkt, ps, last=False)
        osb = stage.tile([65, TQ], f32, tag="o")

        def out_upper():
            nc.vector.tensor_copy(osb[:, 512:], pv[:, 512:])
            nc.sync.dma_start(out=outT[:, 512:], in_=osb[:, 512:])

        pkt, ps = pipe.pop(0)
        attend_rest(pkt, ps, last=True, between=out_upper)
        nc.vector.tensor_copy(osb[:, 0:512], pv[:, 0:512])
        nc.sync.dma_start(out=outT[:, 0:512], in_=osb[:, 0:512])

    nc.compile()
    return nc


def _prep_inputs(x, Wq, Wk, Wv, np_dt):
    """Per-core input maps."""
    wkv = np.empty((128, 8, 128), dtype=np_dt)
    wkv[:, :, 0:64] = Wk.reshape(8, 128, 64).transpose(1, 0, 2)
    wkv[:, :, 64:128] = Wv.reshape(8, 128, 64).transpose(1, 0, 2)
    wq = np.ascontiguousarray(
        Wq.reshape(8, 128, 64).transpose(1, 0, 2)).astype(np_dt)

    s = np.arange(T)
    p_idx = np.arange(128)[:, None]
    col = np.arange(32)[None, :]

    in_maps = []
    for core in range(NCORES):
        b, c = divmod(core, 4)
        # column roll: slot s <- abs column 4*(s//4) + ((s%4 + c) % 4)
        perm = 4 * (s // 4) + ((s % 4 + c) % 4)
        xT = np.ascontiguousarray(x[b].T[:, perm]).astype(np_dt)
        # band mask: key slot p (within its tile) visible to band column col?
        abs_k = 4 * (p_idx // 4) + ((p_idx % 4 + c) % 4)
        abs_q = 4 * col + c
        m32 = (abs_k <= abs_q).astype(np_dt)
        in_maps.append({
            "xT": xT,
            "wkv": wkv,
            "wq": wq,
            "m32": m32,
        })
    return in_maps


def kernel(x, Wq, Wk, Wv, _trace=False, _trace_cores=None):
    from concourse.bass_utils import run_bass_kernel_spmd

    dt_x, np_dt = _dtypes()

    key = ("prog", str(dt_x))
    if key not in _CACHE:
        _CACHE[key] = _build_program(dt_x)
    nc = _CACHE[key]

    in_maps = _prep_inputs(
        np.asarray(x, np.float32), np.asarray(Wq, np.float32),
        np.asarray(Wk, np.float32), np.asarray(Wv, np.float32), np_dt)

    res = run_bass_kernel_spmd(
        nc, in_maps, core_ids=list(range(NCORES)), trace=_trace,
        trace_cores=_trace_cores)

    jidx = 4 * np.arange(TQ)
    out = np.empty((B, T, D), dtype=np.float32)
    for core in range(NCORES):
        b, c = divmod(core, 4)
        o = res.results[core]["outT"]  # [65, TQ]
        out[b, jidx + c, :] = (o[0:64, :] / o[64:65, :]).T
    if _trace:
        return out, res
    return out
